# revision 22
# baseline (speedup 1.0000x reference)
"""Trainium2 Bass kernel for nn_Head_84043920048318 (sparse_attention).

Reference computation (per batch b):
    q = x @ Wq; k = x @ Wk; v = x @ Wv           [T, HS]
    wei = (q @ k.T) * C**-0.5                    [T, T]
    for s:  P = softmax(wei * adjacent[b, s], axis=-1);  out[b, s] = P @ v

Sharding: data-parallel over B across 8 NeuronCores (4 batches each);
projection weights replicated.

Host staging (layout only):
  - adjacent shipped pre-transposed to [b, s, p(u%128), ub, t] bf16
  - x shipped pre-transposed as xT [b, c, t]
  - kernel returns flash-attention-style partials: unnormalized out +
    softmax denominator (col 128), partition-major [b, p, s, tb, 129] bf16;
    final divide + un-transpose on host.

Per-core dataflow (all in transposed [u, t] layout; ACT runs only exp,
which is the critical engine; batch preambles are software-pipelined into
the middle of the previous batch so the exp stream never stalls):
  - projections qT/kT [h, t] (f32r), v natural [u, d] + ones column
  - weiT[u, t] = k @ q.T via matmul(lhsT=kT_block, rhs=qT)
  - per (b, s): prodT = adjT * weiT (DVE bf16 2x; some slices on GpSimd),
    ptT = exp(SCALE*prodT) (ACT), AV matmuls with ptT stationary (FWL)
    against [v | 1] into tb-paired PSUM banks, DVE strided copy -> bf16,
    per-s output DMA.

exp without max-subtraction is safe: |scale * wei * adj| <~ 8.
"""

import numpy as np
import ml_dtypes

B, S, T, C, HS = 32, 8, 512, 128, 128
NCORES = 8
BPC = B // NCORES
TB = T // 128
UB = T // 128
SCALE = float(C) ** -0.5
AVP = 136   # padded av row pitch (keeps matmul psum offsets 32B-aligned)

GPSIMD_MUL_S = ()          # gpsimd mul offload: net loss (shared SBUF port
                           # contention makes concurrent DVE TT ops 3x slower)
PRE_DMA_AT_S = 0           # hoist next batch's DMAs at this s of current batch
PRE_CMP_AT_S = 3           # hoist next batch's projections at this s
ACT_EGRESS_S = (1, 4, 7)   # slices whose PSUM->SBUF egress runs on ACT
# av psum layout: 4 tb groups in one 2-bank tile; offsets keep each matmul
# output (129 fp32 = 516B) inside a single bank and 32B-aligned
AV_OFF = (0, 136, 512, 648)

_CACHED = None


def _build_module():
    import concourse.bacc as bacc
    import concourse.mybir as mybir
    from concourse import tile

    f32 = mybir.dt.float32
    f32r = mybir.dt.float32r
    bf16 = mybir.dt.bfloat16

    nc = bacc.Bacc("TRN2", target_bir_lowering=False, debug=False, num_devices=1)

    x_d = nc.dram_tensor("xT", [BPC, C, T], f32, kind="ExternalInput").ap()
    adj_d = nc.dram_tensor(
        "adjT", [BPC, S, 128, UB, T], bf16, kind="ExternalInput"
    ).ap()
    wq_d = nc.dram_tensor("Wq", [C, HS], f32, kind="ExternalInput").ap()
    wk_d = nc.dram_tensor("Wk", [C, HS], f32, kind="ExternalInput").ap()
    wv_d = nc.dram_tensor("Wv", [C, HS], f32, kind="ExternalInput").ap()
    out_d = nc.dram_tensor(
        "out", [BPC, 128, S, TB, HS + 1], bf16, kind="ExternalOutput"
    ).ap()

    with tile.TileContext(nc) as tc:
        with (
            tc.tile_pool(name="consts", bufs=1) as consts,
            tc.tile_pool(name="bpool", bufs=2) as bpool,
            tc.tile_pool(name="adjp", bufs=2) as adjp,
            tc.tile_pool(name="spool", bufs=4) as spool,
            tc.tile_pool(name="pav", bufs=3, space="PSUM") as pav,
            tc.tile_pool(name="psmall", bufs=2, space="PSUM") as psmall,
        ):
            wq_sb = consts.tile([C, HS], f32, tag="wq")
            wk_sb = consts.tile([C, HS], f32, tag="wk")
            wv_sb = consts.tile([C, HS], f32, tag="wv")
            nc.sync.dma_start(wq_sb[:], wq_d)
            nc.sync.dma_start(wk_sb[:], wk_d)
            nc.sync.dma_start(wv_sb[:], wv_d)

            st = {}  # per-batch live tiles

            def pre_dma(b):
                xT = bpool.tile([C, T], f32, tag="xT")
                nc.sync.dma_start(xT[:], x_d[b])
                adjT = adjp.tile([128, S, UB, T], bf16, tag="adjT")
                for s in range(S):
                    nc.sync.dma_start(adjT[:, s], adj_d[b, s])
                st[b] = {"xT": xT, "adjT": adjT}

            def pre_compute(b):
                xT = st[b]["xT"]
                qT_ps = psmall.tile([HS, T], f32, tag="ps")
                nc.tensor.matmul(qT_ps[:], wq_sb[:], xT[:])
                qT = bpool.tile([HS, T], f32r, tag="qT")
                nc.vector.tensor_copy(qT[:], qT_ps[:])

                kT_ps = psmall.tile([HS, T], f32, tag="ps")
                nc.tensor.matmul(kT_ps[:], wk_sb[:], xT[:])
                kT = bpool.tile([HS, T], f32r, tag="kT")
                nc.vector.tensor_copy(kT[:], kT_ps[:])

                vp = bpool.tile([128, UB, HS + 1], bf16, tag="vp")
                v_ps = psmall.tile([HS, T], f32, tag="ps")
                for ub in range(UB):
                    nc.tensor.matmul(
                        v_ps[:, ub * 128 : (ub + 1) * 128],
                        xT[:, ub * 128 : (ub + 1) * 128],
                        wv_sb[:],
                        start=True,
                        stop=True,
                    )
                nc.vector.tensor_copy(
                    vp[:, :, 0:HS],
                    v_ps[:].rearrange("p (ub q) -> p ub q", ub=UB),
                )
                nc.vector.memset(vp[:, :, HS : HS + 1], 1.0)

                weiT = bpool.tile([128, UB, T], bf16, tag="weiT")
                for ub in range(UB):
                    w_ps = psmall.tile([128, T], f32, tag="ps")
                    nc.tensor.matmul(
                        w_ps[:], kT[:, ub * 128 : (ub + 1) * 128], qT[:]
                    )
                    nc.vector.tensor_copy(weiT[:, ub, :], w_ps[:])

                outb = bpool.tile([128, S, TB, HS + 1], bf16, tag="outb")
                st[b].update({"vp": vp, "weiT": weiT, "outb": outb})

            def do_slice(b, s):
                adjT, weiT = st[b]["adjT"], st[b]["weiT"]
                vp, outb = st[b]["vp"], st[b]["outb"]
                prod = spool.tile([128, UB, T], bf16, tag="prod")
                nc.vector.tensor_mul(prod[:], adjT[:, s], weiT[:])
                pt = spool.tile([128, UB, T], bf16, tag="pt")
                nc.scalar.activation(
                    pt[:], prod[:], mybir.ActivationFunctionType.Exp,
                    scale=SCALE,
                )
                av4 = pav.tile([128, 1024], f32, tag="av4")
                for tb in range(TB):
                    off = AV_OFF[tb]
                    for ub in range(UB):
                        nc.tensor.matmul(
                            av4[:, off : off + HS + 1],
                            pt[:, ub, tb * 128 : (tb + 1) * 128],
                            vp[:, ub, :],
                            start=(ub == 0),
                            stop=(ub == UB - 1),
                        )
                # unnormalized out + denominator, PSUM -> SBUF bf16, one op
                av_view = av4[:].rearrange("p (h q) -> p h q", h=2)[
                    :, :, 0:272
                ].rearrange("p h (tbo q) -> p h tbo q", tbo=2)[
                    :, :, :, 0 : HS + 1
                ]
                out_view = outb[:, s].rearrange("p (h tbo) q -> p h tbo q", h=2)
                if s in ACT_EGRESS_S:
                    nc.scalar.copy(out_view, av_view)
                else:
                    nc.vector.tensor_copy(out_view, av_view)
                nc.sync.dma_start(out_d[b][:, s], outb[:, s])

            pre_dma(0)
            pre_compute(0)
            for b in range(BPC):
                for s in range(S):
                    if b + 1 < BPC:
                        if s == PRE_DMA_AT_S:
                            pre_dma(b + 1)
                        if s == PRE_CMP_AT_S:
                            pre_compute(b + 1)
                    do_slice(b, s)

    nc.compile()
    return nc


def _get_module():
    global _CACHED
    if _CACHED is None:
        _CACHED = _build_module()
    return _CACHED


def run_on_hw(in_maps, trace=False, trace_kwargs=None):
    """Run the compiled module on the 8 NeuronCores. Returns BassKernelResults."""
    from concourse.bass_utils import run_bass_kernel_spmd
    from concourse.bass_interp import get_hw_module

    nc = _get_module()
    old_m = nc.m
    nc.m = get_hw_module(nc.m)
    try:
        return run_bass_kernel_spmd(
            nc,
            in_maps,
            core_ids=list(range(NCORES)),
            trace=trace,
            **(trace_kwargs or {}),
        )
    finally:
        nc.m = old_m


def make_in_maps(x, adjacent, Wq, Wk, Wv):
    bf16 = ml_dtypes.bfloat16
    x = np.ascontiguousarray(x, dtype=np.float32)
    # xT: [B, C, T]
    xT = np.ascontiguousarray(x.transpose(0, 2, 1))
    # adjT: [B, S, p(u%128), ub, t]  bf16
    adjT = np.ascontiguousarray(
        np.asarray(adjacent, dtype=np.float32)
        .transpose(0, 1, 3, 2)           # [b, s, u, t]
        .reshape(B, S, UB, 128, T)       # u -> (ub, p)
        .transpose(0, 1, 3, 2, 4)        # [b, s, p, ub, t]
        .astype(bf16)
    )
    Wq = np.ascontiguousarray(Wq, dtype=np.float32)
    Wk = np.ascontiguousarray(Wk, dtype=np.float32)
    Wv = np.ascontiguousarray(Wv, dtype=np.float32)
    return [
        {
            "xT": xT[c * BPC : (c + 1) * BPC],
            "adjT": adjT[c * BPC : (c + 1) * BPC],
            "Wq": Wq,
            "Wk": Wk,
            "Wv": Wv,
        }
        for c in range(NCORES)
    ]


def kernel(**inputs) -> np.ndarray:
    in_maps = make_in_maps(
        inputs["x"], inputs["adjacent"], inputs["Wq"], inputs["Wk"], inputs["Wv"]
    )
    res = run_on_hw(in_maps)
    # out: per-core [BPC, 128, S, TB, HS+1] bf16 (unnormalized + denom col)
    outs = []
    for c in range(NCORES):
        o = np.asarray(res.results[c]["out"], dtype=np.float32)
        o = o.transpose(0, 2, 3, 1, 4)                # [BPC, S, TB, 128, HS+1]
        o = o.reshape(BPC, S, T, HS + 1)
        outs.append(o[..., :HS] / o[..., HS:])
    return np.ascontiguousarray(np.concatenate(outs, axis=0), dtype=np.float32)


# revision 26
# speedup vs baseline: 1.0395x; 1.0395x over previous
"""Trainium2 Bass kernel for nn_Head_84043920048318 (sparse_attention).

Reference computation (per batch b):
    q = x @ Wq; k = x @ Wk; v = x @ Wv           [T, HS]
    wei = (q @ k.T) * C**-0.5                    [T, T]
    for s:  P = softmax(wei * adjacent[b, s], axis=-1);  out[b, s] = P @ v

Sharding: data-parallel over B across 8 NeuronCores (4 batches each);
projection weights replicated.

Host staging (layout only):
  - adjacent shipped pre-transposed to [b, s, p(u%128), ub, t] bf16
  - x shipped pre-transposed as xT [b, c, t]
  - kernel returns flash-attention-style partials: unnormalized out +
    softmax denominator (col 128), partition-major [b, p, s, tb, 129] bf16;
    final divide + un-transpose on host.

Per-core dataflow (all in transposed [u, t] layout; ACT runs only exp,
which is the critical engine; batch preambles are software-pipelined into
the middle of the previous batch so the exp stream never stalls):
  - projections qT/kT [h, t] (f32r), v natural [u, d] + ones column
  - weiT[u, t] = k @ q.T via matmul(lhsT=kT_block, rhs=qT)
  - per (b, s): prodT = adjT * weiT (DVE bf16 2x; some slices on GpSimd),
    ptT = exp(SCALE*prodT) (ACT), AV matmuls with ptT stationary (FWL)
    against [v | 1] into tb-paired PSUM banks, DVE strided copy -> bf16,
    per-s output DMA.

exp without max-subtraction is safe: |scale * wei * adj| <~ 8.
"""

import numpy as np
import ml_dtypes

B, S, T, C, HS = 32, 8, 512, 128, 128
NCORES = 8
BPC = B // NCORES
TB = T // 128
UB = T // 128
SCALE = float(C) ** -0.5
AVP = 136   # padded av row pitch (keeps matmul psum offsets 32B-aligned)

GPSIMD_MUL_S = ()          # gpsimd mul offload: net loss (shared SBUF port
                           # contention makes concurrent DVE TT ops 3x slower)
PRE_DMA_AT_S = 0           # hoist next batch's DMAs at this s of current batch
PRE_CMP_AT_S = 3           # hoist next batch's projections at this s
ACT_EGRESS_S = (1, 4, 7)   # slices whose PSUM->SBUF egress runs on ACT
# av psum layout: 4 tb groups in one 2-bank tile; offsets keep each matmul
# output (129 fp32 = 516B) inside a single bank and 32B-aligned
AV_OFF = (0, 136, 512, 648)

_CACHED = None


def _build_module():
    import concourse.bacc as bacc
    import concourse.mybir as mybir
    from concourse import tile

    f32 = mybir.dt.float32
    f32r = mybir.dt.float32r
    bf16 = mybir.dt.bfloat16

    nc = bacc.Bacc("TRN2", target_bir_lowering=False, debug=False, num_devices=1)

    x_d = nc.dram_tensor("xT", [BPC, C, T], f32, kind="ExternalInput").ap()
    adj_d = nc.dram_tensor(
        "adjT", [BPC, S, 128, UB, T], bf16, kind="ExternalInput"
    ).ap()
    wq_d = nc.dram_tensor("Wq", [C, HS], f32, kind="ExternalInput").ap()
    wk_d = nc.dram_tensor("Wk", [C, HS], f32, kind="ExternalInput").ap()
    wv_d = nc.dram_tensor("Wv", [C, HS], f32, kind="ExternalInput").ap()
    out_d = nc.dram_tensor(
        "out", [BPC, 128, S, TB, HS + 1], bf16, kind="ExternalOutput"
    ).ap()

    with tile.TileContext(nc) as tc:
        with (
            tc.tile_pool(name="consts", bufs=1) as consts,
            tc.tile_pool(name="bpool", bufs=2) as bpool,
            tc.tile_pool(name="adjp", bufs=2) as adjp,
            tc.tile_pool(name="spool", bufs=4) as spool,
            tc.tile_pool(name="pav", bufs=3, space="PSUM") as pav,
            tc.tile_pool(name="psmall", bufs=2, space="PSUM") as psmall,
        ):
            wq_sb = consts.tile([C, HS], f32, tag="wq")
            wk_sb = consts.tile([C, HS], f32, tag="wk")
            wv_sb = consts.tile([C, HS], f32, tag="wv")
            nc.sync.dma_start(wq_sb[:], wq_d)
            nc.sync.dma_start(wk_sb[:], wk_d)
            nc.sync.dma_start(wv_sb[:], wv_d)

            st = {}  # per-batch live tiles

            def pre_dma(b):
                xT = bpool.tile([C, T], f32, tag="xT")
                nc.sync.dma_start(xT[:], x_d[b])
                adjT = adjp.tile([128, S, UB, T], bf16, tag="adjT")
                for s in range(S):
                    nc.sync.dma_start(adjT[:, s], adj_d[b, s])
                st[b] = {"xT": xT, "adjT": adjT}

            def pre_compute(b):
                xT = st[b]["xT"]
                qT_ps = psmall.tile([HS, T], f32, tag="ps")
                nc.tensor.matmul(qT_ps[:], wq_sb[:], xT[:])
                qT = bpool.tile([HS, T], f32r, tag="qT")
                nc.scalar.copy(qT[:], qT_ps[:])

                kT_ps = psmall.tile([HS, T], f32, tag="ps")
                nc.tensor.matmul(kT_ps[:], wk_sb[:], xT[:])
                kT = bpool.tile([HS, T], f32r, tag="kT")
                nc.scalar.copy(kT[:], kT_ps[:])

                vp = bpool.tile([128, UB, HS + 1], bf16, tag="vp")
                v_ps = psmall.tile([HS, T], f32, tag="ps")
                for ub in range(UB):
                    nc.tensor.matmul(
                        v_ps[:, ub * 128 : (ub + 1) * 128],
                        xT[:, ub * 128 : (ub + 1) * 128],
                        wv_sb[:],
                        start=True,
                        stop=True,
                    )
                nc.scalar.copy(
                    vp[:, :, 0:HS],
                    v_ps[:].rearrange("p (ub q) -> p ub q", ub=UB),
                )
                nc.vector.memset(vp[:, :, HS : HS + 1], 1.0)

                weiT = bpool.tile([128, UB, T], bf16, tag="weiT")
                for ub in range(UB):
                    w_ps = psmall.tile([128, T], f32, tag="ps")
                    nc.tensor.matmul(
                        w_ps[:], kT[:, ub * 128 : (ub + 1) * 128], qT[:]
                    )
                    nc.vector.tensor_copy(weiT[:, ub, :], w_ps[:])

                outb = bpool.tile([128, S, TB, HS + 1], bf16, tag="outb")
                st[b].update({"vp": vp, "weiT": weiT, "outb": outb})

            def do_slice(b, s):
                adjT, weiT = st[b]["adjT"], st[b]["weiT"]
                vp, outb = st[b]["vp"], st[b]["outb"]
                prod = spool.tile([128, UB, T], bf16, tag="prod")
                nc.vector.tensor_mul(prod[:], adjT[:, s], weiT[:])
                pt = spool.tile([128, UB, T], bf16, tag="pt")
                nc.scalar.activation(
                    pt[:], prod[:], mybir.ActivationFunctionType.Exp,
                    scale=SCALE,
                )
                av4 = pav.tile([128, 1024], f32, tag="av4")
                for tb in range(TB):
                    off = AV_OFF[tb]
                    for ub in range(UB):
                        nc.tensor.matmul(
                            av4[:, off : off + HS + 1],
                            pt[:, ub, tb * 128 : (tb + 1) * 128],
                            vp[:, ub, :],
                            start=(ub == 0),
                            stop=(ub == UB - 1),
                        )
                # unnormalized out + denominator, PSUM -> SBUF bf16, one op
                av_view = av4[:].rearrange("p (h q) -> p h q", h=2)[
                    :, :, 0:272
                ].rearrange("p h (tbo q) -> p h tbo q", tbo=2)[
                    :, :, :, 0 : HS + 1
                ]
                out_view = outb[:, s].rearrange("p (h tbo) q -> p h tbo q", h=2)
                nc.vector.tensor_copy(out_view, av_view)
                if s % 2 == 1:
                    nc.sync.dma_start(
                        out_d[b][:, s - 1 : s + 1], outb[:, s - 1 : s + 1]
                    )

            pre_dma(0)
            pre_compute(0)
            for b in range(BPC):
                for s in range(S):
                    if b + 1 < BPC:
                        if s == PRE_DMA_AT_S:
                            pre_dma(b + 1)
                        if s == PRE_CMP_AT_S:
                            pre_compute(b + 1)
                    do_slice(b, s)

    nc.compile()
    return nc


def _get_module():
    global _CACHED
    if _CACHED is None:
        _CACHED = _build_module()
    return _CACHED


def run_on_hw(in_maps, trace=False, trace_kwargs=None):
    """Run the compiled module on the 8 NeuronCores. Returns BassKernelResults."""
    from concourse.bass_utils import run_bass_kernel_spmd
    from concourse.bass_interp import get_hw_module

    nc = _get_module()
    old_m = nc.m
    nc.m = get_hw_module(nc.m)
    try:
        return run_bass_kernel_spmd(
            nc,
            in_maps,
            core_ids=list(range(NCORES)),
            trace=trace,
            **(trace_kwargs or {}),
        )
    finally:
        nc.m = old_m


def make_in_maps(x, adjacent, Wq, Wk, Wv):
    bf16 = ml_dtypes.bfloat16
    x = np.ascontiguousarray(x, dtype=np.float32)
    # xT: [B, C, T]
    xT = np.ascontiguousarray(x.transpose(0, 2, 1))
    # adjT: [B, S, p(u%128), ub, t]  bf16
    adjT = np.ascontiguousarray(
        np.asarray(adjacent, dtype=np.float32)
        .transpose(0, 1, 3, 2)           # [b, s, u, t]
        .reshape(B, S, UB, 128, T)       # u -> (ub, p)
        .transpose(0, 1, 3, 2, 4)        # [b, s, p, ub, t]
        .astype(bf16)
    )
    Wq = np.ascontiguousarray(Wq, dtype=np.float32)
    Wk = np.ascontiguousarray(Wk, dtype=np.float32)
    Wv = np.ascontiguousarray(Wv, dtype=np.float32)
    return [
        {
            "xT": xT[c * BPC : (c + 1) * BPC],
            "adjT": adjT[c * BPC : (c + 1) * BPC],
            "Wq": Wq,
            "Wk": Wk,
            "Wv": Wv,
        }
        for c in range(NCORES)
    ]


def kernel(**inputs) -> np.ndarray:
    in_maps = make_in_maps(
        inputs["x"], inputs["adjacent"], inputs["Wq"], inputs["Wk"], inputs["Wv"]
    )
    res = run_on_hw(in_maps)
    # out: per-core [BPC, 128, S, TB, HS+1] bf16 (unnormalized + denom col)
    outs = []
    for c in range(NCORES):
        o = np.asarray(res.results[c]["out"], dtype=np.float32)
        o = o.transpose(0, 2, 3, 1, 4)                # [BPC, S, TB, 128, HS+1]
        o = o.reshape(BPC, S, T, HS + 1)
        outs.append(o[..., :HS] / o[..., HS:])
    return np.ascontiguousarray(np.concatenate(outs, axis=0), dtype=np.float32)


# revision 27
# speedup vs baseline: 1.0566x; 1.0165x over previous
"""Trainium2 Bass kernel for nn_Head_84043920048318 (sparse_attention).

Reference computation (per batch b):
    q = x @ Wq; k = x @ Wk; v = x @ Wv           [T, HS]
    wei = (q @ k.T) * C**-0.5                    [T, T]
    for s:  P = softmax(wei * adjacent[b, s], axis=-1);  out[b, s] = P @ v

Sharding: data-parallel over B across 8 NeuronCores (4 batches each);
projection weights replicated.

Host staging (layout only):
  - adjacent shipped pre-transposed to [b, s, p(u%128), ub, t] bf16
  - x shipped pre-transposed as xT [b, c, t]
  - kernel returns flash-attention-style partials: unnormalized out +
    softmax denominator (col 128), partition-major [b, p, s, tb, 129] bf16;
    final divide + un-transpose on host.

Per-core dataflow (all in transposed [u, t] layout; ACT runs only exp,
which is the critical engine; batch preambles are software-pipelined into
the middle of the previous batch so the exp stream never stalls):
  - projections qT/kT [h, t] (f32r), v natural [u, d] + ones column
  - weiT[u, t] = k @ q.T via matmul(lhsT=kT_block, rhs=qT)
  - per (b, s): prodT = adjT * weiT (DVE bf16 2x; some slices on GpSimd),
    ptT = exp(SCALE*prodT) (ACT), AV matmuls with ptT stationary (FWL)
    against [v | 1] into tb-paired PSUM banks, DVE strided copy -> bf16,
    per-s output DMA.

exp without max-subtraction is safe: |scale * wei * adj| <~ 8.
"""

import numpy as np
import ml_dtypes

B, S, T, C, HS = 32, 8, 512, 128, 128
NCORES = 8
BPC = B // NCORES
TB = T // 128
UB = T // 128
SCALE = float(C) ** -0.5
AVP = 136   # padded av row pitch (keeps matmul psum offsets 32B-aligned)

GPSIMD_MUL_S = ()          # gpsimd mul offload: net loss (shared SBUF port
                           # contention makes concurrent DVE TT ops 3x slower)
PRE_DMA_AT_S = 0           # hoist next batch's DMAs at this s of current batch
PRE_CMP_AT_S = 3           # hoist next batch's projections at this s
ACT_EGRESS_S = (1, 4, 7)   # slices whose PSUM->SBUF egress runs on ACT
# av psum layout: 4 tb groups in one 2-bank tile; offsets keep each matmul
# output (129 fp32 = 516B) inside a single bank and 32B-aligned
AV_OFF = (0, 136, 512, 648)

_CACHED = None


def _build_module():
    import concourse.bacc as bacc
    import concourse.mybir as mybir
    from concourse import tile

    f32 = mybir.dt.float32
    f32r = mybir.dt.float32r
    bf16 = mybir.dt.bfloat16

    nc = bacc.Bacc("TRN2", target_bir_lowering=False, debug=False, num_devices=1)

    x_d = nc.dram_tensor("xT", [BPC, C, T], f32, kind="ExternalInput").ap()
    adj_d = nc.dram_tensor(
        "adjT", [BPC, S, 128, UB, T], bf16, kind="ExternalInput"
    ).ap()
    wq_d = nc.dram_tensor("Wq", [C, HS], f32, kind="ExternalInput").ap()
    wk_d = nc.dram_tensor("Wk", [C, HS], f32, kind="ExternalInput").ap()
    wv_d = nc.dram_tensor("Wv", [C, HS], f32, kind="ExternalInput").ap()
    out_d = nc.dram_tensor(
        "out", [BPC, 128, S, TB, HS + 1], bf16, kind="ExternalOutput"
    ).ap()

    with tile.TileContext(nc) as tc:
        with (
            tc.tile_pool(name="consts", bufs=1) as consts,
            tc.tile_pool(name="bpool", bufs=2) as bpool,
            tc.tile_pool(name="adjp", bufs=2) as adjp,
            tc.tile_pool(name="spool", bufs=4) as spool,
            tc.tile_pool(name="pav", bufs=3, space="PSUM") as pav,
            tc.tile_pool(name="psmall", bufs=2, space="PSUM") as psmall,
        ):
            wq_sb = consts.tile([C, HS], f32, tag="wq")
            wk_sb = consts.tile([C, HS], f32, tag="wk")
            wv_sb = consts.tile([C, HS], f32, tag="wv")
            nc.sync.dma_start(wq_sb[:], wq_d)
            nc.sync.dma_start(wk_sb[:], wk_d)
            nc.sync.dma_start(wv_sb[:], wv_d)

            st = {}  # per-batch live tiles

            def pre_dma(b):
                xT = bpool.tile([C, T], f32, tag="xT")
                nc.sync.dma_start(xT[:], x_d[b])
                adjT = adjp.tile([128, S, UB, T], bf16, tag="adjT")
                for s in range(S):
                    nc.sync.dma_start(adjT[:, s], adj_d[b, s])
                st[b] = {"xT": xT, "adjT": adjT}

            def pre_compute(b):
                xT = st[b]["xT"]
                qT_ps = psmall.tile([HS, T], f32, tag="ps")
                nc.tensor.matmul(qT_ps[:], wq_sb[:], xT[:])
                qT = bpool.tile([HS, T], f32r, tag="qT")
                nc.scalar.copy(qT[:], qT_ps[:])

                kT_ps = psmall.tile([HS, T], f32, tag="ps")
                nc.tensor.matmul(kT_ps[:], wk_sb[:], xT[:])
                kT = bpool.tile([HS, T], f32r, tag="kT")
                nc.scalar.copy(kT[:], kT_ps[:])

                vp = bpool.tile([128, UB, HS + 1], bf16, tag="vp")
                v_ps = psmall.tile([HS, T], f32, tag="ps")
                for ub in range(UB):
                    nc.tensor.matmul(
                        v_ps[:, ub * 128 : (ub + 1) * 128],
                        xT[:, ub * 128 : (ub + 1) * 128],
                        wv_sb[:],
                        start=True,
                        stop=True,
                    )
                nc.scalar.copy(
                    vp[:, :, 0:HS],
                    v_ps[:].rearrange("p (ub q) -> p ub q", ub=UB),
                )
                nc.vector.memset(vp[:, :, HS : HS + 1], 1.0)

                weiT = bpool.tile([128, UB, T], bf16, tag="weiT")
                for ub in range(UB):
                    w_ps = psmall.tile([128, T], f32, tag="ps")
                    nc.tensor.matmul(
                        w_ps[:], kT[:, ub * 128 : (ub + 1) * 128], qT[:]
                    )
                    nc.vector.tensor_copy(weiT[:, ub, :], w_ps[:])

                outb = bpool.tile([128, S, TB, HS + 1], bf16, tag="outb")
                st[b].update({"vp": vp, "weiT": weiT, "outb": outb})

            pending = []  # (b, s, av4) awaiting egress — lagged one slice so
                          # the PSUM->SBUF copy never head-of-line blocks the
                          # next multiply in the DVE FIFO

            def flush_egress():
                pb, ps, av4 = pending.pop(0)
                outb = st[pb]["outb"]
                # unnormalized out + denominator, PSUM -> SBUF bf16, one op
                av_view = av4[:].rearrange("p (h q) -> p h q", h=2)[
                    :, :, 0:272
                ].rearrange("p h (tbo q) -> p h tbo q", tbo=2)[
                    :, :, :, 0 : HS + 1
                ]
                out_view = outb[:, ps].rearrange(
                    "p (h tbo) q -> p h tbo q", h=2
                )
                nc.vector.tensor_copy(out_view, av_view)
                nc.sync.dma_start(out_d[pb][:, ps], outb[:, ps])

            def do_slice(b, s):
                adjT, weiT = st[b]["adjT"], st[b]["weiT"]
                vp = st[b]["vp"]
                prod = spool.tile([128, UB, T], bf16, tag="prod")
                nc.vector.tensor_mul(prod[:], adjT[:, s], weiT[:])
                pt = spool.tile([128, UB, T], bf16, tag="pt")
                nc.scalar.activation(
                    pt[:], prod[:], mybir.ActivationFunctionType.Exp,
                    scale=SCALE,
                )
                av4 = pav.tile([128, 1024], f32, tag="av4")
                for tb in range(TB):
                    off = AV_OFF[tb]
                    for ub in range(UB):
                        nc.tensor.matmul(
                            av4[:, off : off + HS + 1],
                            pt[:, ub, tb * 128 : (tb + 1) * 128],
                            vp[:, ub, :],
                            start=(ub == 0),
                            stop=(ub == UB - 1),
                        )
                pending.append((b, s, av4))
                if len(pending) > 1:
                    flush_egress()

            pre_dma(0)
            pre_compute(0)
            for b in range(BPC):
                for s in range(S):
                    if b + 1 < BPC:
                        if s == PRE_DMA_AT_S:
                            pre_dma(b + 1)
                        if s == PRE_CMP_AT_S:
                            pre_compute(b + 1)
                    do_slice(b, s)
            while pending:
                flush_egress()

    nc.compile()
    return nc


def _get_module():
    global _CACHED
    if _CACHED is None:
        _CACHED = _build_module()
    return _CACHED


def run_on_hw(in_maps, trace=False, trace_kwargs=None):
    """Run the compiled module on the 8 NeuronCores. Returns BassKernelResults."""
    from concourse.bass_utils import run_bass_kernel_spmd
    from concourse.bass_interp import get_hw_module

    nc = _get_module()
    old_m = nc.m
    nc.m = get_hw_module(nc.m)
    try:
        return run_bass_kernel_spmd(
            nc,
            in_maps,
            core_ids=list(range(NCORES)),
            trace=trace,
            **(trace_kwargs or {}),
        )
    finally:
        nc.m = old_m


def make_in_maps(x, adjacent, Wq, Wk, Wv):
    bf16 = ml_dtypes.bfloat16
    x = np.ascontiguousarray(x, dtype=np.float32)
    # xT: [B, C, T]
    xT = np.ascontiguousarray(x.transpose(0, 2, 1))
    # adjT: [B, S, p(u%128), ub, t]  bf16
    adjT = np.ascontiguousarray(
        np.asarray(adjacent, dtype=np.float32)
        .transpose(0, 1, 3, 2)           # [b, s, u, t]
        .reshape(B, S, UB, 128, T)       # u -> (ub, p)
        .transpose(0, 1, 3, 2, 4)        # [b, s, p, ub, t]
        .astype(bf16)
    )
    Wq = np.ascontiguousarray(Wq, dtype=np.float32)
    Wk = np.ascontiguousarray(Wk, dtype=np.float32)
    Wv = np.ascontiguousarray(Wv, dtype=np.float32)
    return [
        {
            "xT": xT[c * BPC : (c + 1) * BPC],
            "adjT": adjT[c * BPC : (c + 1) * BPC],
            "Wq": Wq,
            "Wk": Wk,
            "Wv": Wv,
        }
        for c in range(NCORES)
    ]


def kernel(**inputs) -> np.ndarray:
    in_maps = make_in_maps(
        inputs["x"], inputs["adjacent"], inputs["Wq"], inputs["Wk"], inputs["Wv"]
    )
    res = run_on_hw(in_maps)
    # out: per-core [BPC, 128, S, TB, HS+1] bf16 (unnormalized + denom col)
    outs = []
    for c in range(NCORES):
        o = np.asarray(res.results[c]["out"], dtype=np.float32)
        o = o.transpose(0, 2, 3, 1, 4)                # [BPC, S, TB, 128, HS+1]
        o = o.reshape(BPC, S, T, HS + 1)
        outs.append(o[..., :HS] / o[..., HS:])
    return np.ascontiguousarray(np.concatenate(outs, axis=0), dtype=np.float32)


# revision 32
# speedup vs baseline: 1.0706x; 1.0132x over previous
"""Trainium2 Bass kernel for nn_Head_84043920048318 (sparse_attention).

Reference computation (per batch b):
    q = x @ Wq; k = x @ Wk; v = x @ Wv           [T, HS]
    wei = (q @ k.T) * C**-0.5                    [T, T]
    for s:  P = softmax(wei * adjacent[b, s], axis=-1);  out[b, s] = P @ v

Sharding: data-parallel over B across 8 NeuronCores (4 batches each);
projection weights replicated.

Host staging (layout only):
  - adjacent shipped pre-transposed to [b, s, p(u%128), ub, t] bf16
  - x shipped pre-transposed as xT [b, c, t]
  - kernel returns flash-attention-style partials: unnormalized out +
    softmax denominator (col 128), partition-major [b, p, s, tb, 129] bf16;
    final divide + un-transpose on host.

Per-core dataflow (all in transposed [u, t] layout; ACT runs only exp,
which is the critical engine; batch preambles are software-pipelined into
the middle of the previous batch so the exp stream never stalls):
  - projections qT/kT [h, t] (f32r), v natural [u, d] + ones column
  - weiT[u, t] = k @ q.T via matmul(lhsT=kT_block, rhs=qT)
  - per (b, s): prodT = adjT * weiT (DVE bf16 2x; some slices on GpSimd),
    ptT = exp(SCALE*prodT) (ACT), AV matmuls with ptT stationary (FWL)
    against [v | 1] into tb-paired PSUM banks, DVE strided copy -> bf16,
    per-s output DMA.

exp without max-subtraction is safe: |scale * wei * adj| <~ 8.
"""

import numpy as np
import ml_dtypes

B, S, T, C, HS = 32, 8, 512, 128, 128
NCORES = 8
BPC = B // NCORES
TB = T // 128
UB = T // 128
SCALE = float(C) ** -0.5
AVP = 136   # padded av row pitch (keeps matmul psum offsets 32B-aligned)

GPSIMD_MUL_S = ()          # gpsimd mul offload: net loss (shared SBUF port
                           # contention makes concurrent DVE TT ops 3x slower)
PRE_DMA_AT_S = 0           # hoist next batch's DMAs at this s of current batch
PRE_CMP_AT_S = 3           # hoist next batch's projections at this s
ACT_EGRESS_S = (1, 4, 7)   # slices whose PSUM->SBUF egress runs on ACT
# av psum layout: 4 tb groups in one 2-bank tile; offsets keep each matmul
# output (129 fp32 = 516B) inside a single bank and 32B-aligned
AV_OFF = (0, 136, 512, 648)

_CACHED = None


def _build_module():
    import concourse.bacc as bacc
    import concourse.mybir as mybir
    from concourse import tile

    f32 = mybir.dt.float32
    f32r = mybir.dt.float32r
    bf16 = mybir.dt.bfloat16

    nc = bacc.Bacc("TRN2", target_bir_lowering=False, debug=False, num_devices=1)

    x_d = nc.dram_tensor("xT", [BPC, C, T], f32, kind="ExternalInput").ap()
    adj_d = nc.dram_tensor(
        "adjT", [BPC, S, 128, UB, T], bf16, kind="ExternalInput"
    ).ap()
    w3_d = nc.dram_tensor("W3", [3, C, HS], f32, kind="ExternalInput").ap()
    out_d = nc.dram_tensor(
        "out", [BPC, 128, S, TB, HS + 1], bf16, kind="ExternalOutput"
    ).ap()

    with tile.TileContext(nc) as tc:
        with (
            tc.tile_pool(name="consts", bufs=1) as consts,
            tc.tile_pool(name="bpool", bufs=2) as bpool,
            tc.tile_pool(name="adjp", bufs=2) as adjp,
            tc.tile_pool(name="spool", bufs=4) as spool,
            tc.tile_pool(name="pav", bufs=3, space="PSUM") as pav,
            tc.tile_pool(name="psmall", bufs=2, space="PSUM") as psmall,
        ):
            st = {}  # per-batch live tiles

            def pre_dma(b):
                xT = bpool.tile([C, T], f32, tag="xT")
                nc.sync.dma_start(xT[:], x_d[b])
                adjT = adjp.tile([128, S, UB, T], bf16, tag="adjT")
                for s in range(0, S, 2):
                    nc.sync.dma_start(
                        adjT[:, s : s + 2],
                        adj_d[b, s : s + 2].rearrange("s p ub t -> p s ub t"),
                    )
                st[b] = {"xT": xT, "adjT": adjT}

            # batch-0 fill: xT first (it gates the projection chain), then the
            # weights in a single packed transfer, then batch-0 adjacency
            xT0 = bpool.tile([C, T], f32, tag="xT")
            nc.sync.dma_start(xT0[:], x_d[0])
            w3_sb = consts.tile([C, 3, HS], f32, tag="w3")
            nc.sync.dma_start(w3_sb[:], w3_d.rearrange("w c h -> c w h"))
            wq_sb, wk_sb, wv_sb = w3_sb[:, 0], w3_sb[:, 1], w3_sb[:, 2]
            adjT0 = adjp.tile([128, S, UB, T], bf16, tag="adjT")
            for s0 in range(0, S, 2):
                nc.sync.dma_start(
                    adjT0[:, s0 : s0 + 2],
                    adj_d[0, s0 : s0 + 2].rearrange("s p ub t -> p s ub t"),
                )
            st[0] = {"xT": xT0, "adjT": adjT0}

            def pre_compute(b):
                xT = st[b]["xT"]
                qT_ps = psmall.tile([HS, T], f32, tag="ps")
                nc.tensor.matmul(qT_ps[:], wq_sb, xT[:])
                qT = bpool.tile([HS, T], f32r, tag="qT")
                nc.scalar.copy(qT[:], qT_ps[:])

                kT_ps = psmall.tile([HS, T], f32, tag="ps")
                nc.tensor.matmul(kT_ps[:], wk_sb, xT[:])
                kT = bpool.tile([HS, T], f32r, tag="kT")
                nc.scalar.copy(kT[:], kT_ps[:])

                vp = bpool.tile([128, UB, HS + 1], bf16, tag="vp")
                v_ps = psmall.tile([HS, T], f32, tag="ps")
                for ub in range(UB):
                    nc.tensor.matmul(
                        v_ps[:, ub * 128 : (ub + 1) * 128],
                        xT[:, ub * 128 : (ub + 1) * 128],
                        wv_sb,
                        start=True,
                        stop=True,
                    )
                nc.scalar.copy(
                    vp[:, :, 0:HS],
                    v_ps[:].rearrange("p (ub q) -> p ub q", ub=UB),
                )
                nc.vector.memset(vp[:, :, HS : HS + 1], 1.0)

                weiT = bpool.tile([128, UB, T], bf16, tag="weiT")
                for ub in range(UB):
                    w_ps = psmall.tile([128, T], f32, tag="ps")
                    nc.tensor.matmul(
                        w_ps[:], kT[:, ub * 128 : (ub + 1) * 128], qT[:]
                    )
                    nc.vector.tensor_copy(weiT[:, ub, :], w_ps[:])

                outb = bpool.tile([128, S, TB, HS + 1], bf16, tag="outb")
                st[b].update({"vp": vp, "weiT": weiT, "outb": outb})

            pending = []  # (b, s, av4) awaiting egress — lagged one slice so
                          # the PSUM->SBUF copy never head-of-line blocks the
                          # next multiply in the DVE FIFO

            def flush_egress():
                pb, ps, av4 = pending.pop(0)
                outb = st[pb]["outb"]
                # unnormalized out + denominator, PSUM -> SBUF bf16, one op
                av_view = av4[:].rearrange("p (h q) -> p h q", h=2)[
                    :, :, 0:272
                ].rearrange("p h (tbo q) -> p h tbo q", tbo=2)[
                    :, :, :, 0 : HS + 1
                ]
                out_view = outb[:, ps].rearrange(
                    "p (h tbo) q -> p h tbo q", h=2
                )
                nc.vector.tensor_copy(out_view, av_view)
                nc.sync.dma_start(out_d[pb][:, ps], outb[:, ps])

            def do_slice(b, s):
                adjT, weiT = st[b]["adjT"], st[b]["weiT"]
                vp = st[b]["vp"]
                prod = spool.tile([128, UB, T], bf16, tag="prod")
                nc.vector.tensor_mul(prod[:], adjT[:, s], weiT[:])
                pt = spool.tile([128, UB, T], bf16, tag="pt")
                nc.scalar.activation(
                    pt[:], prod[:], mybir.ActivationFunctionType.Exp,
                    scale=SCALE,
                )
                av4 = pav.tile([128, 1024], f32, tag="av4")
                for tb in range(TB):
                    off = AV_OFF[tb]
                    for ub in range(UB):
                        nc.tensor.matmul(
                            av4[:, off : off + HS + 1],
                            pt[:, ub, tb * 128 : (tb + 1) * 128],
                            vp[:, ub, :],
                            start=(ub == 0),
                            stop=(ub == UB - 1),
                        )
                pending.append((b, s, av4))
                if len(pending) > 1:
                    flush_egress()

            pre_compute(0)
            for b in range(BPC):
                for s in range(S):
                    if b + 1 < BPC:
                        if s == PRE_DMA_AT_S:
                            pre_dma(b + 1)
                        if s == PRE_CMP_AT_S:
                            pre_compute(b + 1)
                    do_slice(b, s)
            while pending:
                flush_egress()

    nc.compile()
    return nc


def _get_module():
    global _CACHED
    if _CACHED is None:
        _CACHED = _build_module()
    return _CACHED


def run_on_hw(in_maps, trace=False, trace_kwargs=None):
    """Run the compiled module on the 8 NeuronCores. Returns BassKernelResults."""
    from concourse.bass_utils import run_bass_kernel_spmd
    from concourse.bass_interp import get_hw_module

    nc = _get_module()
    old_m = nc.m
    nc.m = get_hw_module(nc.m)
    try:
        return run_bass_kernel_spmd(
            nc,
            in_maps,
            core_ids=list(range(NCORES)),
            trace=trace,
            **(trace_kwargs or {}),
        )
    finally:
        nc.m = old_m


def make_in_maps(x, adjacent, Wq, Wk, Wv):
    bf16 = ml_dtypes.bfloat16
    x = np.ascontiguousarray(x, dtype=np.float32)
    # xT: [B, C, T]
    xT = np.ascontiguousarray(x.transpose(0, 2, 1))
    # adjT: [B, S, p(u%128), ub, t]  bf16
    adjT = np.ascontiguousarray(
        np.asarray(adjacent, dtype=np.float32)
        .transpose(0, 1, 3, 2)           # [b, s, u, t]
        .reshape(B, S, UB, 128, T)       # u -> (ub, p)
        .transpose(0, 1, 3, 2, 4)        # [b, s, p, ub, t]
        .astype(bf16)
    )
    W3 = np.ascontiguousarray(
        np.stack([Wq, Wk, Wv]).astype(np.float32)
    )
    return [
        {
            "xT": xT[c * BPC : (c + 1) * BPC],
            "adjT": adjT[c * BPC : (c + 1) * BPC],
            "W3": W3,
        }
        for c in range(NCORES)
    ]


def kernel(**inputs) -> np.ndarray:
    in_maps = make_in_maps(
        inputs["x"], inputs["adjacent"], inputs["Wq"], inputs["Wk"], inputs["Wv"]
    )
    res = run_on_hw(in_maps)
    # out: per-core [BPC, 128, S, TB, HS+1] bf16 (unnormalized + denom col)
    outs = []
    for c in range(NCORES):
        o = np.asarray(res.results[c]["out"], dtype=np.float32)
        o = o.transpose(0, 2, 3, 1, 4)                # [BPC, S, TB, 128, HS+1]
        o = o.reshape(BPC, S, T, HS + 1)
        outs.append(o[..., :HS] / o[..., HS:])
    return np.ascontiguousarray(np.concatenate(outs, axis=0), dtype=np.float32)


# revision 35
# speedup vs baseline: 1.1198x; 1.0460x over previous
"""Trainium2 Bass kernel for nn_Head_84043920048318 (sparse_attention).

Reference computation (per batch b):
    q = x @ Wq; k = x @ Wk; v = x @ Wv           [T, HS]
    wei = (q @ k.T) * C**-0.5                    [T, T]
    for s:  P = softmax(wei * adjacent[b, s], axis=-1);  out[b, s] = P @ v

Sharding: data-parallel over B across 8 NeuronCores (4 batches each);
projection weights replicated.

Host staging (layout only):
  - adjacent shipped pre-transposed to [b, s, p(u%128), ub, t] bf16
  - x shipped pre-transposed as xT [b, c, t]
  - kernel returns flash-attention-style partials: unnormalized out +
    softmax denominator (col 128), partition-major [b, p, s, tb, 129] bf16;
    final divide + un-transpose on host.

Per-core dataflow (all in transposed [u, t] layout; ACT runs only exp,
which is the critical engine; batch preambles are software-pipelined into
the middle of the previous batch so the exp stream never stalls):
  - projections qT/kT [h, t] (f32r), v natural [u, d] + ones column
  - weiT[u, t] = k @ q.T via matmul(lhsT=kT_block, rhs=qT)
  - per (b, s): prodT = adjT * weiT (DVE bf16 2x; some slices on GpSimd),
    ptT = exp(SCALE*prodT) (ACT), AV matmuls with ptT stationary (FWL)
    against [v | 1] into tb-paired PSUM banks, DVE strided copy -> bf16,
    per-s output DMA.

exp without max-subtraction is safe: |scale * wei * adj| <~ 8.
"""

import numpy as np
import ml_dtypes

B, S, T, C, HS = 32, 8, 512, 128, 128
NCORES = 8
BPC = B // NCORES
TB = T // 128
UB = T // 128
SCALE = float(C) ** -0.5
AVP = 136   # padded av row pitch (keeps matmul psum offsets 32B-aligned)

GPSIMD_MUL_S = ()          # gpsimd mul offload: net loss (shared SBUF port
                           # contention makes concurrent DVE TT ops 3x slower)
PRE_DMA_AT_S = 0           # hoist next batch's DMAs at this s of current batch
PRE_CMP_AT_S = 3           # hoist next batch's projections at this s
ACT_EGRESS_S = (1, 4, 7)   # slices whose PSUM->SBUF egress runs on ACT
# av psum layout: 4 tb groups in one 2-bank tile; offsets keep each matmul
# output (129 fp32 = 516B) inside a single bank and 32B-aligned
AV_OFF = (0, 136, 512, 648)

_CACHED = None


def _build_module():
    import concourse.bacc as bacc
    import concourse.mybir as mybir
    from concourse import tile

    f32 = mybir.dt.float32
    f32r = mybir.dt.float32r
    bf16 = mybir.dt.bfloat16

    nc = bacc.Bacc("TRN2", target_bir_lowering=False, debug=False, num_devices=1)

    x_d = nc.dram_tensor("xT", [BPC, C, T], f32, kind="ExternalInput").ap()
    adj_d = nc.dram_tensor(
        "adjT", [BPC, S, 128, UB, T], bf16, kind="ExternalInput"
    ).ap()
    w3_d = nc.dram_tensor("W3", [3, C, HS], f32, kind="ExternalInput").ap()
    out_d = nc.dram_tensor(
        "out", [BPC, 128, S, TB, HS + 1], bf16, kind="ExternalOutput"
    ).ap()

    with tile.TileContext(nc) as tc:
        with (
            tc.tile_pool(name="consts", bufs=1) as consts,
            tc.tile_pool(name="bpool", bufs=2) as bpool,
            tc.tile_pool(name="adjp", bufs=2) as adjp,
            tc.tile_pool(name="spool", bufs=4) as spool,
            tc.tile_pool(name="pav", bufs=2, space="PSUM") as pav,
            tc.tile_pool(name="psmall", bufs=3, space="PSUM") as psmall,
        ):
            st = {}  # per-batch live tiles

            def pre_dma(b):
                xT = bpool.tile([C, T], f32, tag="xT")
                nc.sync.dma_start(xT[:], x_d[b])
                adjT = adjp.tile([128, S, UB, T], bf16, tag="adjT")
                for s in range(0, S, 2):
                    nc.sync.dma_start(
                        adjT[:, s : s + 2],
                        adj_d[b, s : s + 2].rearrange("s p ub t -> p s ub t"),
                    )
                st[b] = {"xT": xT, "adjT": adjT}

            # batch-0 fill: xT first (it gates the projection chain), then the
            # weights in a single packed transfer, then batch-0 adjacency
            xT0 = bpool.tile([C, T], f32, tag="xT")
            nc.sync.dma_start(xT0[:], x_d[0])
            w3_sb = consts.tile([C, 3, HS], f32, tag="w3")
            nc.sync.dma_start(w3_sb[:], w3_d.rearrange("w c h -> c w h"))
            wq_sb, wk_sb, wv_sb = w3_sb[:, 0], w3_sb[:, 1], w3_sb[:, 2]
            adjT0 = adjp.tile([128, S, UB, T], bf16, tag="adjT")
            for s0 in range(0, S, 2):
                nc.sync.dma_start(
                    adjT0[:, s0 : s0 + 2],
                    adj_d[0, s0 : s0 + 2].rearrange("s p ub t -> p s ub t"),
                )
            st[0] = {"xT": xT0, "adjT": adjT0}

            def pre_mm(b):
                # projection matmuls only — the copies are emitted two slices
                # later (pre_rest) so they are long-ready when ACT/DVE reach
                # them and never head-of-line block the exp/mul streams
                xT = st[b]["xT"]
                qT_ps = psmall.tile([HS, T], f32, tag="ps")
                nc.tensor.matmul(qT_ps[:], wq_sb, xT[:])
                kT_ps = psmall.tile([HS, T], f32, tag="ps")
                nc.tensor.matmul(kT_ps[:], wk_sb, xT[:])
                v_ps = psmall.tile([HS, T], f32, tag="ps")
                for ub in range(UB):
                    nc.tensor.matmul(
                        v_ps[:, ub * 128 : (ub + 1) * 128],
                        xT[:, ub * 128 : (ub + 1) * 128],
                        wv_sb,
                        start=True,
                        stop=True,
                    )
                st[b].update({"qT_ps": qT_ps, "kT_ps": kT_ps, "v_ps": v_ps})

            def pre_rest(b):
                qT = bpool.tile([HS, T], f32r, tag="qT")
                nc.scalar.copy(qT[:], st[b]["qT_ps"][:])
                kT = bpool.tile([HS, T], f32r, tag="kT")
                nc.scalar.copy(kT[:], st[b]["kT_ps"][:])
                vp = bpool.tile([128, UB, HS + 1], bf16, tag="vp")
                nc.scalar.copy(
                    vp[:, :, 0:HS],
                    st[b]["v_ps"][:].rearrange("p (ub q) -> p ub q", ub=UB),
                )
                nc.vector.memset(vp[:, :, HS : HS + 1], 1.0)

                weiT = bpool.tile([128, UB, T], bf16, tag="weiT")
                for ub in range(UB):
                    w_ps = psmall.tile([128, T], f32, tag="ps")
                    nc.tensor.matmul(
                        w_ps[:], kT[:, ub * 128 : (ub + 1) * 128], qT[:]
                    )
                    nc.vector.tensor_copy(weiT[:, ub, :], w_ps[:])

                outb = bpool.tile([128, S, TB, HS + 1], bf16, tag="outb")
                st[b].update({"vp": vp, "weiT": weiT, "outb": outb})

            def pre_compute(b):
                pre_mm(b)
                pre_rest(b)

            pending = []  # (b, s, av4) awaiting egress — lagged one slice so
                          # the PSUM->SBUF copy never head-of-line blocks the
                          # next multiply in the DVE FIFO

            def flush_egress():
                pb, ps, av4 = pending.pop(0)
                outb = st[pb]["outb"]
                # unnormalized out + denominator, PSUM -> SBUF bf16, one op
                av_view = av4[:].rearrange("p (h q) -> p h q", h=2)[
                    :, :, 0:272
                ].rearrange("p h (tbo q) -> p h tbo q", tbo=2)[
                    :, :, :, 0 : HS + 1
                ]
                out_view = outb[:, ps].rearrange(
                    "p (h tbo) q -> p h tbo q", h=2
                )
                nc.vector.tensor_copy(out_view, av_view)
                nc.sync.dma_start(out_d[pb][:, ps], outb[:, ps])

            def do_slice(b, s):
                adjT, weiT = st[b]["adjT"], st[b]["weiT"]
                vp = st[b]["vp"]
                prod = spool.tile([128, UB, T], bf16, tag="prod")
                nc.vector.tensor_mul(prod[:], adjT[:, s], weiT[:])
                pt = spool.tile([128, UB, T], bf16, tag="pt")
                nc.scalar.activation(
                    pt[:], prod[:], mybir.ActivationFunctionType.Exp,
                    scale=SCALE,
                )
                av4 = pav.tile([128, 1024], f32, tag="av4")
                for tb in range(TB):
                    off = AV_OFF[tb]
                    for ub in range(UB):
                        nc.tensor.matmul(
                            av4[:, off : off + HS + 1],
                            pt[:, ub, tb * 128 : (tb + 1) * 128],
                            vp[:, ub, :],
                            start=(ub == 0),
                            stop=(ub == UB - 1),
                        )
                pending.append((b, s, av4))
                if len(pending) > 1:
                    flush_egress()

            pre_compute(0)
            for b in range(BPC):
                for s in range(S):
                    if b + 1 < BPC:
                        if s == PRE_DMA_AT_S:
                            pre_dma(b + 1)
                        if s == 1:
                            pre_mm(b + 1)
                        if s == PRE_CMP_AT_S:
                            pre_rest(b + 1)
                    do_slice(b, s)
            while pending:
                flush_egress()

    nc.compile()
    return nc


def _get_module():
    global _CACHED
    if _CACHED is None:
        _CACHED = _build_module()
    return _CACHED


def run_on_hw(in_maps, trace=False, trace_kwargs=None):
    """Run the compiled module on the 8 NeuronCores. Returns BassKernelResults."""
    from concourse.bass_utils import run_bass_kernel_spmd
    from concourse.bass_interp import get_hw_module

    nc = _get_module()
    old_m = nc.m
    nc.m = get_hw_module(nc.m)
    try:
        return run_bass_kernel_spmd(
            nc,
            in_maps,
            core_ids=list(range(NCORES)),
            trace=trace,
            **(trace_kwargs or {}),
        )
    finally:
        nc.m = old_m


def make_in_maps(x, adjacent, Wq, Wk, Wv):
    bf16 = ml_dtypes.bfloat16
    x = np.ascontiguousarray(x, dtype=np.float32)
    # xT: [B, C, T]
    xT = np.ascontiguousarray(x.transpose(0, 2, 1))
    # adjT: [B, S, p(u%128), ub, t]  bf16
    adjT = np.ascontiguousarray(
        np.asarray(adjacent, dtype=np.float32)
        .transpose(0, 1, 3, 2)           # [b, s, u, t]
        .reshape(B, S, UB, 128, T)       # u -> (ub, p)
        .transpose(0, 1, 3, 2, 4)        # [b, s, p, ub, t]
        .astype(bf16)
    )
    W3 = np.ascontiguousarray(
        np.stack([Wq, Wk, Wv]).astype(np.float32)
    )
    return [
        {
            "xT": xT[c * BPC : (c + 1) * BPC],
            "adjT": adjT[c * BPC : (c + 1) * BPC],
            "W3": W3,
        }
        for c in range(NCORES)
    ]


def kernel(**inputs) -> np.ndarray:
    in_maps = make_in_maps(
        inputs["x"], inputs["adjacent"], inputs["Wq"], inputs["Wk"], inputs["Wv"]
    )
    res = run_on_hw(in_maps)
    # out: per-core [BPC, 128, S, TB, HS+1] bf16 (unnormalized + denom col)
    outs = []
    for c in range(NCORES):
        o = np.asarray(res.results[c]["out"], dtype=np.float32)
        o = o.transpose(0, 2, 3, 1, 4)                # [BPC, S, TB, 128, HS+1]
        o = o.reshape(BPC, S, T, HS + 1)
        outs.append(o[..., :HS] / o[..., HS:])
    return np.ascontiguousarray(np.concatenate(outs, axis=0), dtype=np.float32)
